# revision 1
# baseline (speedup 1.0000x reference)
"""Trainium2 Bass kernel for the batched kinematics layer.

Math:
  Per batch element b: root transform Tg(qpos[b,0:6]) via Rodrigues; then per
  chain c the sequential composition T <- T @ (P0[c,j] + sin(q)*P1 + cos(q)*P2)
  where P0/P1/P2 are constant 4x4s precomputed on host from offsets/axes
  (P0 = off + off@K2h, P1 = off@Kh, P2 = -off@K2h).  The per-link vertex
  transform pts = R@v + t is one matmul per link with contraction K=12:
  out[b, (v,x)] = sum_k A[k,b] * W[k,(v,x)], A = transposed link-transform
  entries (k = x*4+l), W built on host from verts (zeros + copies only).

  sin/cos go through the ScalarE Sin LUT, which is only accurate on ~[-pi,pi],
  so inputs are range-reduced with x - 2pi*round(x/2pi) (fp32 magic-number
  rounding).

Sharding: pure data-parallel over batch, 8 cores x 512 batch elements.
"""
import math
import numpy as np
from contextlib import ExitStack

import concourse.bass as bass
import concourse.mybir as mybir
import concourse.tile as tile
from concourse import bacc
from concourse.bass_utils import run_bass_kernel_spmd
from concourse.masks import make_identity

F32 = mybir.dt.float32
F32R = mybir.dt.float32r
BF16 = mybir.dt.bfloat16
AX = mybir.AxisListType
OP = mybir.AluOpType
AF = mybir.ActivationFunctionType

N_CHAINS, N_JOINTS, N_VERTS = 5, 4, 512
NLINK = N_CHAINS * N_JOINTS          # 20
VX = N_VERTS * 3                     # 1536
ROW = NLINK * VX                     # 30720
B_FULL = 4096
N_CORES = 8
B_CORE = B_FULL // N_CORES           # 512
P = 128
NB = B_CORE // P                     # 4 batch tiles per core
EPS = 1e-8
TWO_PI = float(np.float32(2.0 * math.pi))
INV_2PI = float(np.float32(1.0 / (2.0 * math.pi)))
MAGIC = 12582912.0                   # 1.5 * 2**23: fp32 round-to-nearest trick

# matmul mode for the big vertex-transform matmuls:
#   "pack" : bf16 hi/lo split packed into one K=36 matmul (~fp32-accurate,
#            full PE speed: lhsT=[Ah;Al;Ah], rhs=[Wh;Wh;Wl])
#   "f32r" : relaxed-precision fp32 (full PE speed at N>=256, ~1.6e-4 err)
#   "f32"  : exact fp32 (4 cyc/row on PE)
#   "hilo" : bf16 hi/lo split, 3 accumulated matmuls (~fp32-accurate)
MM_MODE = "pack"
REPEAT = 1


def _view(t, off, dims):
    """Custom free-dim view of a tile AP: keep partition pair, replace free dims."""
    ap = [list(t.ap[0])] + [[s, c] for (s, c) in dims]
    return bass.AP(t.tensor, t.offset + off, ap)


def _host_constants(offsets, axes, verts):
    off = offsets.astype(np.float64)
    ax = axes.astype(np.float64)
    K = np.zeros((N_CHAINS, N_JOINTS, 4, 4))
    x, y, z = ax[..., 0], ax[..., 1], ax[..., 2]
    K[..., 0, 1] = -z; K[..., 0, 2] = y
    K[..., 1, 0] = z;  K[..., 1, 2] = -x
    K[..., 2, 0] = -y; K[..., 2, 1] = x
    K2 = K @ K
    offK = off @ K
    offK2 = off @ K2
    pcon = np.stack([off + offK2, offK, -offK2], 0).reshape(3, NLINK, 16)
    pcon = np.ascontiguousarray(pcon, np.float32)

    W = np.zeros((12, NLINK, VX), np.float32)
    vv = verts.reshape(NLINK, N_VERTS, 3)
    for xx in range(3):
        for l in range(3):
            W[xx * 4 + l, :, xx::3] = vv[:, :, l]
        W[xx * 4 + 3, :, xx::3] = 1.0
    return pcon, W


def _build_nc(mm_mode, repeat):
    import ml_dtypes
    nc = bacc.Bacc("TRN2", target_bir_lowering=False, debug=False)

    qpos = nc.dram_tensor("qpos", [B_CORE, 26], F32, kind="ExternalInput")
    pcon = nc.dram_tensor("pcon", [3 * NLINK * 16], F32, kind="ExternalInput")
    if mm_mode == "hilo":
        wmat = nc.dram_tensor("wmat", [2, 12, NLINK * VX], BF16, kind="ExternalInput")
    elif mm_mode == "pack":
        wmat = nc.dram_tensor("wmat", [36, NLINK * VX], BF16, kind="ExternalInput")
    else:
        MMDT = F32R if mm_mode == "f32r" else F32
        wmat = nc.dram_tensor("wmat", [12, NLINK * VX], MMDT, kind="ExternalInput")
    out = nc.dram_tensor("out", [B_CORE, ROW], F32, kind="ExternalOutput")

    with tile.TileContext(nc) as tc, ExitStack() as ctx:
        const = ctx.enter_context(tc.tile_pool(name="const", bufs=1))
        qp_pool = ctx.enter_context(tc.tile_pool(name="qp", bufs=2))
        small = ctx.enter_context(tc.tile_pool(name="small", bufs=2))
        tpool = ctx.enter_context(tc.tile_pool(name="tpool", bufs=6))
        mpool = ctx.enter_context(tc.tile_pool(name="mpool", bufs=2))
        apool = ctx.enter_context(tc.tile_pool(name="apool", bufs=8))
        ostage = ctx.enter_context(tc.tile_pool(name="ostage", bufs=3))
        psA = ctx.enter_context(tc.tile_pool(name="psA", bufs=4, space="PSUM"))
        psO = ctx.enter_context(tc.tile_pool(name="psO", bufs=4, space="PSUM"))

        # ---- constants ----
        ident = const.tile([P, P], F32, name="ident")
        make_identity(nc, ident)
        ident_bf = const.tile([P, P], BF16, name="ident_bf")
        make_identity(nc, ident_bf)

        pt = const.tile([P, 3 * NLINK * 16], F32, name="pt")  # broadcast P0/P1/P2
        nc.gpsimd.dma_start(out=pt, in_=bass.AP(pcon, 0, [[0, P], [1, 3 * NLINK * 16]]))

        if mm_mode == "hilo":
            wh_sb = const.tile([12, NLINK * VX], BF16, name="wh_sb")
            nc.sync.dma_start(out=wh_sb, in_=wmat[0])
            wl_sb = const.tile([12, NLINK * VX], BF16, name="wl_sb")
            nc.sync.dma_start(out=wl_sb, in_=wmat[1])
        elif mm_mode == "pack":
            w_sb = const.tile([36, NLINK * VX], BF16, name="w_sb")
            nc.sync.dma_start(out=w_sb, in_=wmat[:])
        else:
            w_sb = const.tile([12, NLINK * VX], wmat.dtype, name="w_sb")
            nc.sync.dma_start(out=w_sb, in_=wmat[:])

        zero_c = const.tile([P, 1], F32, name="zero_c")
        nc.vector.memset(zero_c, 0.0)

        loop_ctx = tc.For_i(0, repeat, 1) if repeat > 1 else None
        if loop_ctx is not None:
            ctx.enter_context(loop_ctx)
        if True:
            for bt in range(NB):
                # ---- load qpos tile ----
                qp = qp_pool.tile([P, 26], F32, name="qp")
                nc.sync.dma_start(out=qp, in_=qpos[bt * P:(bt + 1) * P, :])

                # ---- root angle ----
                sq = small.tile([P, 3], F32, name="sq")
                nc.vector.tensor_mul(sq, qp[:, 3:6], qp[:, 3:6])
                s2 = small.tile([P, 1], F32, name="s2")
                nc.vector.tensor_reduce(s2, sq, AX.X, OP.add)
                ang = small.tile([P, 1], F32, name="ang")
                nc.scalar.activation(ang, s2, AF.Sqrt, bias=zero_c)
                angc = small.tile([P, 1], F32, name="angc")
                nc.vector.tensor_scalar_max(angc, ang, EPS)
                inv = small.tile([P, 1], F32, name="inv")
                nc.vector.reciprocal(inv, angc)
                axs = small.tile([P, 3], F32, name="axs")
                nc.vector.tensor_scalar_mul(axs, qp[:, 3:6], inv)

                # ---- range-reduced sin/cos of [q(20), root_angle] ----
                scin = small.tile([P, 21], F32, name="scin")
                nc.vector.tensor_copy(scin[:, 0:20], qp[:, 6:26])
                nc.vector.tensor_copy(scin[:, 20:21], ang)

                def reduced_sin(dst_name, src_ap, phase):
                    # returns tile with sin(src + phase), range-reduced
                    xin = small.tile([P, 21], F32, name=dst_name + "_x")
                    if phase == 0.0:
                        nc.vector.tensor_copy(xin, src_ap)
                    else:
                        nc.vector.tensor_scalar_add(xin, src_ap, phase)
                    y = small.tile([P, 21], F32, name=dst_name + "_y")
                    nc.vector.tensor_scalar_mul(y, xin, INV_2PI)
                    nc.vector.tensor_scalar_add(y, y, MAGIC)
                    nc.vector.tensor_scalar_add(y, y, -MAGIC)
                    nc.vector.tensor_scalar_mul(y, y, TWO_PI)
                    nc.vector.tensor_sub(xin, xin, y)
                    sv = small.tile([P, 21], F32, name=dst_name)
                    nc.scalar.activation(sv, xin, AF.Sin, bias=zero_c)
                    return sv

                sinv = reduced_sin("sinv", scin, 0.0)
                cosv = reduced_sin("cosv", scin, math.pi / 2)
                s_r = _view(sinv, 20, [(1, 1)])
                c_r = _view(cosv, 20, [(1, 1)])

                # ---- root transform Tg [P, 12] (cols x*4+m) ----
                omc = small.tile([P, 1], F32, name="omc")
                nc.vector.tensor_scalar(omc, c_r, -1.0, 1.0, OP.mult, OP.add)
                outer = small.tile([P, 9], F32, name="outer")
                nc.vector.tensor_mul(
                    _view(outer, 0, [(3, 3), (1, 3)]),
                    _view(axs, 0, [(1, 3), (0, 3)]),
                    _view(axs, 0, [(0, 3), (1, 3)]),
                )
                Tg = small.tile([P, 12], F32, name="Tg")
                nc.vector.tensor_scalar_mul(
                    _view(Tg, 0, [(4, 3), (1, 3)]),
                    _view(outer, 0, [(3, 3), (1, 3)]),
                    omc,
                )
                nc.vector.tensor_scalar_add(
                    _view(Tg, 0, [(5, 3)]), _view(Tg, 0, [(5, 3)]), c_r
                )
                sa = small.tile([P, 3], F32, name="sa")
                nc.vector.tensor_scalar_mul(sa, axs, s_r)
                for (col, k, op) in ((1, 2, OP.subtract), (2, 1, OP.add),
                                     (4, 2, OP.add), (6, 0, OP.subtract),
                                     (8, 1, OP.subtract), (9, 0, OP.add)):
                    v = _view(Tg, col, [(1, 1)])
                    nc.vector.tensor_tensor(v, v, _view(sa, k, [(1, 1)]), op)
                nc.vector.tensor_copy(_view(Tg, 3, [(4, 3)]), qp[:, 0:3])

                # ---- chain composition (all joints up-front so the DVE
                # critical path isn't queued behind stage-2 PSUM copies) ----
                T_prev = None
                copy_i = 0
                T_list = []
                TL_list = []
                for j in range(N_JOINTS):
                    # M = P0 + s*P1 + c*P2   [P, 5, 16]
                    M = mpool.tile([P, 80], F32, name="M")
                    Mv = _view(M, 0, [(16, 5), (1, 16)])
                    P0v = _view(pt, 0 + j * 16, [(64, 5), (1, 16)])
                    P1v = _view(pt, 320 + j * 16, [(64, 5), (1, 16)])
                    P2v = _view(pt, 640 + j * 16, [(64, 5), (1, 16)])
                    sv = _view(sinv, j, [(4, 5), (0, 16)])
                    cv = _view(cosv, j, [(4, 5), (0, 16)])
                    nc.vector.tensor_mul(Mv, P1v, sv)
                    nc.vector.tensor_add(Mv, Mv, P0v)
                    Mt = mpool.tile([P, 80], F32, name="Mt")
                    Mtv = _view(Mt, 0, [(16, 5), (1, 16)])
                    nc.vector.tensor_mul(Mtv, P2v, cv)
                    nc.vector.tensor_add(Mv, Mv, Mtv)

                    # T_new[c,x,l] = sum_m T_prev[c,x,m]*M[c,m,l] (+T_prev[c,x,3] @ l=3)
                    T_new = tpool.tile([P, 60], F32, name="T_new")
                    Tnv = _view(T_new, 0, [(12, 5), (4, 3), (1, 4)])
                    Ttmp = tpool.tile([P, 60], F32, name="Ttmp", tag="Ttmp")
                    Ttv = _view(Ttmp, 0, [(12, 5), (4, 3), (1, 4)])

                    def prev_view(m):
                        if T_prev is None:
                            return _view(Tg, m, [(0, 5), (4, 3), (0, 4)])
                        return _view(T_prev, m, [(12, 5), (4, 3), (0, 4)])

                    def m_view(m):
                        return _view(M, m * 4, [(16, 5), (0, 3), (1, 4)])

                    nc.vector.tensor_mul(Tnv, prev_view(0), m_view(0))
                    nc.vector.tensor_mul(Ttv, prev_view(1), m_view(1))
                    nc.vector.tensor_add(Tnv, Tnv, Ttv)
                    nc.vector.tensor_mul(Ttv, prev_view(2), m_view(2))
                    nc.vector.tensor_add(Tnv, Tnv, Ttv)
                    t3o = _view(T_new, 3, [(12, 5), (4, 3)])
                    if T_prev is None:
                        t3i = _view(Tg, 3, [(0, 5), (4, 3)])
                    else:
                        t3i = _view(T_prev, 3, [(12, 5), (4, 3)])
                    nc.vector.tensor_tensor(t3o, t3o, t3i, OP.add)
                    T_prev = T_new
                    T_list.append(T_new)

                    if mm_mode == "pack":
                        # bf16 hi/lo split of T in [b, entries] layout, packed
                        # per chain as contiguous [Ah(12) | Al(12) | Ah(12)]
                        TL = tpool.tile([P, 180], BF16, name="TL", tag="TL")
                        hi0 = _view(TL, 0, [(36, 5), (1, 12)])
                        lo = _view(TL, 12, [(36, 5), (1, 12)])
                        hi2 = _view(TL, 24, [(36, 5), (1, 12)])
                        tnv = _view(T_new, 0, [(12, 5), (1, 12)])
                        nc.vector.tensor_copy(hi0, tnv)
                        nc.vector.tensor_tensor(lo, tnv, hi0, OP.subtract)
                        nc.vector.tensor_copy(hi2, hi0)
                        TL_list.append(TL)

                # ---- stage 2: chain-outer so 4 links land contiguous in
                # DRAM and ship as one 3MB DMA ----
                for c in range(N_CHAINS):
                    ot4 = ostage.tile([P, N_JOINTS * VX], F32, name="ot4")
                    for j in range(N_JOINTS):
                        T_new = T_list[j]
                        TL = TL_list[j] if mm_mode == "pack" else None
                        link = c * N_JOINTS + j
                        if mm_mode == "pack":
                            At_ps = psA.tile([36, P], BF16, name="At_ps", space="PSUM")
                            nc.tensor.transpose(At_ps, _view(TL, c * 36, [(1, 36)]), ident_bf)
                            A36 = apool.tile([36, P], BF16, name="A36")
                            nc.vector.tensor_copy(A36, At_ps)
                        else:
                            At_ps = psA.tile([12, P], F32, name="At_ps", space="PSUM")
                            nc.tensor.transpose(At_ps, _view(T_new, c * 12, [(1, 12)]), ident)

                        if mm_mode == "hilo":
                            Ah = apool.tile([12, P], BF16, name="Ah")
                            nc.vector.tensor_copy(Ah, At_ps)
                            Al = apool.tile([12, P], BF16, name="Al")
                            nc.vector.tensor_sub(Al, At_ps, Ah)
                        elif mm_mode != "pack":
                            A_sb = apool.tile([12, P], wmat.dtype, name="A_sb")
                            nc.vector.tensor_copy(A_sb, At_ps)

                        for i in range(3):
                            O_ps = psO.tile([P, 512], F32, name="O_ps", space="PSUM")
                            if mm_mode == "pack":
                                wv = _view(w_sb, link * VX + i * 512, [(1, 512)])
                                nc.tensor.matmul(O_ps, A36[:, :], wv)
                            elif mm_mode == "hilo":
                                whv = _view(wh_sb, link * VX + i * 512, [(1, 512)])
                                wlv = _view(wl_sb, link * VX + i * 512, [(1, 512)])
                                nc.tensor.matmul(O_ps, Ah[:, :], whv,
                                                 start=True, stop=False)
                                nc.tensor.matmul(O_ps, Al[:, :], whv,
                                                 start=False, stop=False)
                                nc.tensor.matmul(O_ps, Ah[:, :], wlv,
                                                 start=False, stop=True)
                            else:
                                wv = _view(w_sb, link * VX + i * 512, [(1, 512)])
                                nc.tensor.matmul(O_ps, A_sb[:, :], wv)
                            oslc = ot4[:, j * VX + i * 512: j * VX + (i + 1) * 512]
                            if copy_i % 3 == 0:
                                nc.vector.tensor_copy(oslc, O_ps)
                            else:
                                nc.scalar.copy(oslc, O_ps)
                            copy_i += 1

                    dst = bass.AP(out, (bt * P) * ROW + c * N_JOINTS * VX,
                                  [[ROW, P], [1, N_JOINTS * VX]])
                    nc.sync.dma_start(out=dst, in_=ot4)

    nc.compile()
    return nc


_NC_CACHE = {}


def _get_nc(mm_mode=None, repeat=None):
    mm_mode = MM_MODE if mm_mode is None else mm_mode
    repeat = REPEAT if repeat is None else repeat
    key = (mm_mode, repeat)
    if key not in _NC_CACHE:
        _NC_CACHE[key] = _build_nc(mm_mode, repeat)
    return _NC_CACHE[key]


def _make_in_maps(qpos, offsets, axes, verts, mm_mode):
    import ml_dtypes
    qpos = np.ascontiguousarray(qpos, np.float32)
    pcon, W = _host_constants(np.asarray(offsets, np.float32),
                              np.asarray(axes, np.float32),
                              np.asarray(verts, np.float32))
    pcon_flat = np.ascontiguousarray(pcon.reshape(-1))
    W = np.ascontiguousarray(W.reshape(12, NLINK * VX))
    if mm_mode == "hilo":
        Wh = W.astype(ml_dtypes.bfloat16)
        Wl = (W - Wh.astype(np.float32)).astype(ml_dtypes.bfloat16)
        Wm = np.ascontiguousarray(np.stack([Wh, Wl], 0))
    elif mm_mode == "pack":
        Wh = W.astype(ml_dtypes.bfloat16)
        Wl = (W - Wh.astype(np.float32)).astype(ml_dtypes.bfloat16)
        Wm = np.ascontiguousarray(np.concatenate([Wh, Wh, Wl], 0))
    else:
        Wm = W
    return [
        {"qpos": np.ascontiguousarray(qpos[i * B_CORE:(i + 1) * B_CORE]),
         "pcon": pcon_flat, "wmat": Wm}
        for i in range(N_CORES)
    ]


def kernel(qpos, offsets, axes, verts):
    nc = _get_nc()
    in_maps = _make_in_maps(qpos, offsets, axes, verts, MM_MODE)
    res = run_bass_kernel_spmd(nc, in_maps, core_ids=list(range(N_CORES)))
    outs = [res.results[i]["out"] for i in range(N_CORES)]
    full = np.concatenate(outs, axis=0)
    return full.reshape(B_FULL, N_CHAINS, N_JOINTS, N_VERTS, 3)



# revision 2
# speedup vs baseline: 1.3236x; 1.3236x over previous
"""Trainium2 Bass kernel for the batched kinematics layer.

Math:
  Per batch element b: root transform Tg(qpos[b,0:6]) via Rodrigues; then per
  chain c the sequential composition T <- T @ (P0[c,j] + sin(q)*P1 + cos(q)*P2)
  where P0/P1/P2 are constant 4x4s precomputed on host from offsets/axes.
  The per-link vertex transform pts = R@v + t is a matmul with contraction
  over the 12 link-transform entries.  Links of one chain are fused into a
  single K=48 contraction against a block-diagonal weight matrix
  W48[48, 4*VX] per chain (built on host from verts), so each chain needs
  only ONE [128,48] transpose of its stacked transform entries.

  sin/cos go through the ScalarE Sin LUT, which is only accurate on ~[-pi,pi],
  so inputs are range-reduced with x - 2pi*round(x/2pi) (fp32 magic-number
  rounding).

Precision: the correctness gate is rel-err(absmax) < 2e-2, so the device
  computes the big matmuls in bf16 and stores the output as bf16 (halving
  the HBM write volume, which is the roofline); the host upcasts to f32.

Sharding: pure data-parallel over batch, 8 cores x 512 batch elements.
"""
import math
import numpy as np
from contextlib import ExitStack

import concourse.bass as bass
import concourse.mybir as mybir
import concourse.tile as tile
from concourse import bacc
from concourse.bass_utils import run_bass_kernel_spmd
from concourse.masks import make_identity

F32 = mybir.dt.float32
BF16 = mybir.dt.bfloat16
AX = mybir.AxisListType
OP = mybir.AluOpType
AF = mybir.ActivationFunctionType

N_CHAINS, N_JOINTS, N_VERTS = 5, 4, 512
NLINK = N_CHAINS * N_JOINTS          # 20
VX = N_VERTS * 3                     # 1536
CROW = N_JOINTS * VX                 # 6144 (one chain's output row chunk)
ROW = NLINK * VX                     # 30720
B_FULL = 4096
N_CORES = 8
B_CORE = B_FULL // N_CORES           # 512
P = 128
NB = B_CORE // P                     # 4 batch tiles per core
K48 = 4 * 12                         # stacked contraction per chain
EPS = 1e-8
TWO_PI = float(np.float32(2.0 * math.pi))
INV_2PI = float(np.float32(1.0 / (2.0 * math.pi)))
MAGIC = 12582912.0                   # 1.5 * 2**23: fp32 round-to-nearest trick

REPEAT = 1
# PSUM->SBUF copy engine pattern per batch-tile (20 copies of FD=1536):
# ScalarE is 1.2 GHz vs VectorE 0.96 GHz and VectorE also runs the chain
# composition, so give ScalarE the larger share.
N_ACT_COPIES = 11


def _view(t, off, dims):
    """Custom free-dim view of a tile AP: keep partition pair, replace free dims."""
    ap = [list(t.ap[0])] + [[s, c] for (s, c) in dims]
    return bass.AP(t.tensor, t.offset + off, ap)


def _host_constants(offsets, axes, verts):
    off = offsets.astype(np.float64)
    ax = axes.astype(np.float64)
    K = np.zeros((N_CHAINS, N_JOINTS, 4, 4))
    x, y, z = ax[..., 0], ax[..., 1], ax[..., 2]
    K[..., 0, 1] = -z; K[..., 0, 2] = y
    K[..., 1, 0] = z;  K[..., 1, 2] = -x
    K[..., 2, 0] = -y; K[..., 2, 1] = x
    K2 = K @ K
    offK = off @ K
    offK2 = off @ K2
    pcon = np.stack([off + offK2, offK, -offK2], 0).reshape(3, NLINK, 16)
    pcon = np.ascontiguousarray(pcon, np.float32)

    # W48: per chain c a block-diagonal [48, CROW] so all 4 links contract in
    # one matmul.  Row 12*j + (x*4 + l) holds vert component l (or 1.0 for
    # l=3) of link (c,j), laid out at column j*VX + v*3 + x.
    W48 = np.zeros((K48, ROW), np.float32)
    vv = verts.reshape(NLINK, N_VERTS, 3)
    for c in range(N_CHAINS):
        for j in range(N_JOINTS):
            link = c * N_JOINTS + j
            base = c * CROW + j * VX
            for xx in range(3):
                for l in range(3):
                    W48[12 * j + xx * 4 + l, base + xx::3][:N_VERTS] = vv[link, :, l]
                W48[12 * j + xx * 4 + 3, base + xx::3][:N_VERTS] = 1.0
    return pcon, W48


def _build_nc(repeat):
    nc = bacc.Bacc("TRN2", target_bir_lowering=False, debug=False)

    qpos = nc.dram_tensor("qpos", [B_CORE, 26], F32, kind="ExternalInput")
    pcon = nc.dram_tensor("pcon", [3 * NLINK * 16], F32, kind="ExternalInput")
    wmat = nc.dram_tensor("wmat", [K48, ROW], BF16, kind="ExternalInput")
    out = nc.dram_tensor("out", [B_CORE, ROW], BF16, kind="ExternalOutput")

    with tile.TileContext(nc) as tc, ExitStack() as ctx:
        const = ctx.enter_context(tc.tile_pool(name="const", bufs=1))
        qp_pool = ctx.enter_context(tc.tile_pool(name="qp", bufs=2))
        small = ctx.enter_context(tc.tile_pool(name="small", bufs=2))
        tpool = ctx.enter_context(tc.tile_pool(name="tpool", bufs=6))
        mpool = ctx.enter_context(tc.tile_pool(name="mpool", bufs=2))
        apool = ctx.enter_context(tc.tile_pool(name="apool", bufs=2))
        ostage = ctx.enter_context(tc.tile_pool(name="ostage", bufs=3))
        psA = ctx.enter_context(tc.tile_pool(name="psA", bufs=2, space="PSUM"))
        psO = ctx.enter_context(tc.tile_pool(name="psO", bufs=2, space="PSUM"))

        # ---- constants ----
        ident_bf = const.tile([P, P], BF16, name="ident_bf")
        make_identity(nc, ident_bf)

        pt = const.tile([P, 3 * NLINK * 16], F32, name="pt")  # broadcast P0/P1/P2
        nc.gpsimd.dma_start(out=pt, in_=bass.AP(pcon, 0, [[0, P], [1, 3 * NLINK * 16]]))

        w_sb = const.tile([K48, ROW], BF16, name="w_sb")
        nc.sync.dma_start(out=w_sb, in_=wmat[:])

        zero_c = const.tile([P, 1], F32, name="zero_c")
        nc.vector.memset(zero_c, 0.0)

        loop_ctx = tc.For_i(0, repeat, 1) if repeat > 1 else None
        if loop_ctx is not None:
            ctx.enter_context(loop_ctx)
        if True:
            for bt in range(NB):
                # ---- load qpos tile ----
                qp = qp_pool.tile([P, 26], F32, name="qp")
                nc.sync.dma_start(out=qp, in_=qpos[bt * P:(bt + 1) * P, :])

                # ---- root angle ----
                sq = small.tile([P, 3], F32, name="sq")
                nc.vector.tensor_mul(sq, qp[:, 3:6], qp[:, 3:6])
                s2 = small.tile([P, 1], F32, name="s2")
                nc.vector.tensor_reduce(s2, sq, AX.X, OP.add)
                ang = small.tile([P, 1], F32, name="ang")
                nc.scalar.activation(ang, s2, AF.Sqrt, bias=zero_c)
                angc = small.tile([P, 1], F32, name="angc")
                nc.vector.tensor_scalar_max(angc, ang, EPS)
                inv = small.tile([P, 1], F32, name="inv")
                nc.vector.reciprocal(inv, angc)
                axs = small.tile([P, 3], F32, name="axs")
                nc.vector.tensor_scalar_mul(axs, qp[:, 3:6], inv)

                # ---- range-reduced sin/cos of [q(20), root_angle] ----
                scin = small.tile([P, 21], F32, name="scin")
                nc.vector.tensor_copy(scin[:, 0:20], qp[:, 6:26])
                nc.vector.tensor_copy(scin[:, 20:21], ang)

                def reduced_sin(dst_name, src_ap, phase):
                    # returns tile with sin(src + phase), range-reduced
                    xin = small.tile([P, 21], F32, name=dst_name + "_x")
                    if phase == 0.0:
                        nc.vector.tensor_copy(xin, src_ap)
                    else:
                        nc.vector.tensor_scalar_add(xin, src_ap, phase)
                    y = small.tile([P, 21], F32, name=dst_name + "_y")
                    nc.vector.tensor_scalar_mul(y, xin, INV_2PI)
                    nc.vector.tensor_scalar_add(y, y, MAGIC)
                    nc.vector.tensor_scalar_add(y, y, -MAGIC)
                    nc.vector.tensor_scalar_mul(y, y, TWO_PI)
                    nc.vector.tensor_sub(xin, xin, y)
                    sv = small.tile([P, 21], F32, name=dst_name)
                    nc.scalar.activation(sv, xin, AF.Sin, bias=zero_c)
                    return sv

                sinv = reduced_sin("sinv", scin, 0.0)
                cosv = reduced_sin("cosv", scin, math.pi / 2)
                s_r = _view(sinv, 20, [(1, 1)])
                c_r = _view(cosv, 20, [(1, 1)])

                # ---- root transform Tg [P, 12] (cols x*4+m) ----
                omc = small.tile([P, 1], F32, name="omc")
                nc.vector.tensor_scalar(omc, c_r, -1.0, 1.0, OP.mult, OP.add)
                outer = small.tile([P, 9], F32, name="outer")
                nc.vector.tensor_mul(
                    _view(outer, 0, [(3, 3), (1, 3)]),
                    _view(axs, 0, [(1, 3), (0, 3)]),
                    _view(axs, 0, [(0, 3), (1, 3)]),
                )
                Tg = small.tile([P, 12], F32, name="Tg")
                nc.vector.tensor_scalar_mul(
                    _view(Tg, 0, [(4, 3), (1, 3)]),
                    _view(outer, 0, [(3, 3), (1, 3)]),
                    omc,
                )
                nc.vector.tensor_scalar_add(
                    _view(Tg, 0, [(5, 3)]), _view(Tg, 0, [(5, 3)]), c_r
                )
                sa = small.tile([P, 3], F32, name="sa")
                nc.vector.tensor_scalar_mul(sa, axs, s_r)
                for (col, k, op) in ((1, 2, OP.subtract), (2, 1, OP.add),
                                     (4, 2, OP.add), (6, 0, OP.subtract),
                                     (8, 1, OP.subtract), (9, 0, OP.add)):
                    v = _view(Tg, col, [(1, 1)])
                    nc.vector.tensor_tensor(v, v, _view(sa, k, [(1, 1)]), op)
                nc.vector.tensor_copy(_view(Tg, 3, [(4, 3)]), qp[:, 0:3])

                # ---- chain composition ----
                T_prev = None
                T_list = []
                for j in range(N_JOINTS):
                    # M = P0 + s*P1 + c*P2   [P, 5, 16]
                    M = mpool.tile([P, 80], F32, name="M")
                    Mv = _view(M, 0, [(16, 5), (1, 16)])
                    P0v = _view(pt, 0 + j * 16, [(64, 5), (1, 16)])
                    P1v = _view(pt, 320 + j * 16, [(64, 5), (1, 16)])
                    P2v = _view(pt, 640 + j * 16, [(64, 5), (1, 16)])
                    sv = _view(sinv, j, [(4, 5), (0, 16)])
                    cv = _view(cosv, j, [(4, 5), (0, 16)])
                    nc.vector.tensor_mul(Mv, P1v, sv)
                    nc.vector.tensor_add(Mv, Mv, P0v)
                    Mt = mpool.tile([P, 80], F32, name="Mt")
                    Mtv = _view(Mt, 0, [(16, 5), (1, 16)])
                    nc.vector.tensor_mul(Mtv, P2v, cv)
                    nc.vector.tensor_add(Mv, Mv, Mtv)

                    # T_new[c,x,l] = sum_m T_prev[c,x,m]*M[c,m,l] (+T_prev[c,x,3] @ l=3)
                    T_new = tpool.tile([P, 60], F32, name="T_new")
                    Tnv = _view(T_new, 0, [(12, 5), (4, 3), (1, 4)])
                    Ttmp = tpool.tile([P, 60], F32, name="Ttmp", tag="Ttmp")
                    Ttv = _view(Ttmp, 0, [(12, 5), (4, 3), (1, 4)])

                    def prev_view(m):
                        if T_prev is None:
                            return _view(Tg, m, [(0, 5), (4, 3), (0, 4)])
                        return _view(T_prev, m, [(12, 5), (4, 3), (0, 4)])

                    def m_view(m):
                        return _view(M, m * 4, [(16, 5), (0, 3), (1, 4)])

                    nc.vector.tensor_mul(Tnv, prev_view(0), m_view(0))
                    nc.vector.tensor_mul(Ttv, prev_view(1), m_view(1))
                    nc.vector.tensor_add(Tnv, Tnv, Ttv)
                    nc.vector.tensor_mul(Ttv, prev_view(2), m_view(2))
                    nc.vector.tensor_add(Tnv, Tnv, Ttv)
                    t3o = _view(T_new, 3, [(12, 5), (4, 3)])
                    if T_prev is None:
                        t3i = _view(Tg, 3, [(0, 5), (4, 3)])
                    else:
                        t3i = _view(T_prev, 3, [(12, 5), (4, 3)])
                    nc.vector.tensor_tensor(t3o, t3o, t3i, OP.add)
                    T_prev = T_new
                    T_list.append(T_new)

                # ---- stack all 20 links' transform entries chain-major as
                # bf16: TL48[:, 48c + 12j + e] = T entry e of link (c,j) ----
                TL48 = mpool.tile([P, NLINK * 12], BF16, name="TL48")
                for j in range(N_JOINTS):
                    src = _view(T_list[j], 0, [(12, 5), (1, 12)])
                    dst = _view(TL48, 12 * j, [(48, 5), (1, 12)])
                    nc.vector.tensor_copy(dst, src)

                # ---- transpose to [48, 128] per chain (one PSUM bank) ----
                At_ps = psA.tile([K48, 5 * P], BF16, name="At_ps", space="PSUM")
                for c in range(N_CHAINS):
                    nc.tensor.transpose(
                        At_ps[:, c * P:(c + 1) * P],
                        TL48[:, c * K48:(c + 1) * K48],
                        ident_bf,
                    )
                At_sb = apool.tile([K48, 5 * P], BF16, name="At_sb")
                nc.vector.tensor_copy(At_sb, At_ps)

                # ---- big vertex-transform matmuls: per chain 12 matmuls of
                # [48,128] x [48,512], grouped 3 per 3-bank PSUM tile so each
                # PSUM->SBUF copy moves FD=1536 ----
                copy_i = 0
                for c in range(N_CHAINS):
                    lhsT = At_sb[:, c * P:(c + 1) * P]
                    ot = ostage.tile([P, CROW], BF16, name="ot")
                    for g in range(4):
                        O_ps = psO.tile([P, 3 * 512], F32, name="O_ps", space="PSUM")
                        for i in range(3):
                            col = c * CROW + g * 1536 + i * 512
                            nc.tensor.matmul(
                                O_ps[:, i * 512:(i + 1) * 512],
                                lhsT, w_sb[:, col:col + 512],
                            )
                        oslc = ot[:, g * 1536:(g + 1) * 1536]
                        if copy_i % 20 < N_ACT_COPIES:
                            nc.scalar.copy(oslc, O_ps)
                        else:
                            nc.vector.tensor_copy(oslc, O_ps)
                        copy_i += 1

                    dst = bass.AP(out, (bt * P) * ROW + c * CROW,
                                  [[ROW, P], [1, CROW]])
                    nc.sync.dma_start(out=dst, in_=ot)

    nc.compile()
    return nc


_NC_CACHE = {}


def _get_nc(repeat=None):
    repeat = REPEAT if repeat is None else repeat
    if repeat not in _NC_CACHE:
        _NC_CACHE[repeat] = _build_nc(repeat)
    return _NC_CACHE[repeat]


def _make_in_maps(qpos, offsets, axes, verts):
    import ml_dtypes
    qpos = np.ascontiguousarray(qpos, np.float32)
    pcon, W48 = _host_constants(np.asarray(offsets, np.float32),
                                np.asarray(axes, np.float32),
                                np.asarray(verts, np.float32))
    pcon_flat = np.ascontiguousarray(pcon.reshape(-1))
    Wm = np.ascontiguousarray(W48.astype(ml_dtypes.bfloat16))
    return [
        {"qpos": np.ascontiguousarray(qpos[i * B_CORE:(i + 1) * B_CORE]),
         "pcon": pcon_flat, "wmat": Wm}
        for i in range(N_CORES)
    ]


def kernel(qpos, offsets, axes, verts):
    nc = _get_nc()
    in_maps = _make_in_maps(qpos, offsets, axes, verts)
    res = run_bass_kernel_spmd(nc, in_maps, core_ids=list(range(N_CORES)))
    outs = [res.results[i]["out"] for i in range(N_CORES)]
    full = np.concatenate(outs, axis=0).astype(np.float32)
    return full.reshape(B_FULL, N_CHAINS, N_JOINTS, N_VERTS, 3)
